# revision 10
# baseline (speedup 1.0000x reference)
"""Multi-head attention v3.1 on 8 TRN2 NeuronCores.

Core c = (batch b = c//2, head-half g = c%2): heads [8g, 8g+8) over ALL
2048 queries. Output projection is computed as a PARTIAL product
ao_g @ w_proj[512 g-rows] on each core; the host sums the two partials
per batch and adds the (folded) bias — zero collectives.

vs v2 ((batch, query-half) split): K and V projections are no longer
duplicated across the two cores of a batch (bg matmul work drops from
768 to 512 slots/core) and weight DMA halves.

Schedule (v3.1):
 - pass = (head-pair mt, query-chunk qc), 16 passes; ScalarE exp
   ([128,1024]-free ~1147 ns) paces the inner loop;
 - the next pass's first two scores+exp are HOISTED into the current
   pass's last kt2 step so the ACT queue never drains at a boundary;
 - pass-end pav/den stashes run on DVE (ACT stays pure-exp);
 - output projection is split per query-chunk into a 3-chunk partial
   (staged to SBUF as soon as pairs 0-2 are normalized) + 1-chunk
   finisher, so only proj-fin(qc=3) (8 matmuls) remains after the last
   pass;
 - prologue FIFO order matches the DMA arrival order (x tokens 0-1023
   first) so the PE never waits on late DMA.
"""

import sys

if "/opt/trn_rl_repo" not in sys.path:
    sys.path.insert(0, "/opt/trn_rl_repo")

import numpy as np
import ml_dtypes

import concourse.bass as bass
import concourse.mybir as mybir
from concourse.tile import TileContext
from concourse.bass_utils import run_bass_kernel_spmd

F32 = mybir.dt.float32
BF16 = mybir.dt.bfloat16

B = 4
N = 2048
C = 1024
H = 16
D = 64
SCALE = D**-0.5
NCORES = 8
CT = C // 128  # 8 contraction tiles over C
KT = N // 128  # 16 key tiles
G = 512  # dims per head-group (8 heads x 64)
GP = G // 128  # 4 partition tiles over the group dims
NPAIR = 4  # head pairs per core
NQC = 4  # query chunks of 512
NPASS = NPAIR * NQC  # 16 passes


def _split_sync_waits(nc, max_waits: int = 1) -> int:
    """Walrus rejects TPB instructions with >1 sync-wait; hoist extras onto
    InstNoOps inserted just before, on the same engine."""
    n_split = 0
    for fn in nc.m.functions:
        for block in fn.blocks:
            out = []
            changed = False
            for inst in block.instructions:
                si = getattr(inst, "sync_info", None)
                if si is not None and len(si.on_wait) > max_waits:
                    waits = list(si.on_wait)
                    n_extra = len(waits) - max_waits
                    for i in range(0, n_extra, max_waits):
                        out.append(
                            mybir.InstNoOp(
                                name=f"{inst.name}-sw{i}",
                                sync_info=mybir.SyncInfo(
                                    on_wait=waits[i : i + max_waits], on_update=[]
                                ),
                                bass_nofuse=True,
                                engine=inst.engine,
                            )
                        )
                    inst.sync_info = mybir.SyncInfo(
                        on_wait=waits[n_extra:], on_update=list(si.on_update)
                    )
                    changed = True
                    n_split += 1
                out.append(inst)
            if changed:
                block.instructions = out
    return n_split


def build():
    nc = bass.Bass(target_bir_lowering=False)

    xT_ext = nc.declare_dram_parameter("xT", [C, N], BF16, isOutput=False)
    wq_ext = nc.declare_dram_parameter("w_q", [C, G], BF16, isOutput=False)
    wk_ext = nc.declare_dram_parameter("w_k", [C, G], BF16, isOutput=False)
    wv_ext = nc.declare_dram_parameter("w_v", [C, G], BF16, isOutput=False)
    wp_ext = nc.declare_dram_parameter("w_p", [G, C], BF16, isOutput=False)
    bq_ext = nc.declare_dram_parameter("b_q", [G, 1], F32, isOutput=False)
    bk_ext = nc.declare_dram_parameter("b_k", [G, 1], F32, isOutput=False)
    out_ext = nc.declare_dram_parameter("out", [C, N], BF16, isOutput=True)

    xT_r = xT_ext[:].rearrange("(o p) n -> p o n", p=128)
    wq_r = wq_ext[:].rearrange("(o p) n -> p o n", p=128)
    wk_r = wk_ext[:].rearrange("(o p) n -> p o n", p=128)
    wv_r = wv_ext[:].rearrange("(o p) n -> p o n", p=128)
    wp_r = wp_ext[:].rearrange("(o p) n -> p o n", p=128)
    out_r = out_ext[:].rearrange("(o p) n -> p o n", p=128)

    with TileContext(nc) as tc:
        with (
            tc.tile_pool(name="const", bufs=1) as const,
            tc.tile_pool(name="kq", bufs=2) as kqp,
            tc.tile_pool(name="at", bufs=4) as atp,
            tc.tile_pool(name="ost", bufs=2) as ostp,
            tc.tile_pool(name="ps_s", bufs=2, space="PSUM") as ps_s,
            tc.tile_pool(name="ps_av", bufs=1, space="PSUM") as ps_av,
            tc.tile_pool(name="ps_den", bufs=1, space="PSUM") as ps_den,
            tc.tile_pool(name="ps_bg", bufs=2, space="PSUM") as ps_bg,
        ):
            # ---- constants / big residents -------------------------------
            xT = const.tile([128, CT, N], BF16)
            wq = const.tile([128, CT, G], BF16)
            wk = const.tile([128, CT, G], BF16)
            wv = const.tile([128, CT, G], BF16)
            wp = const.tile([128, GP, C], BF16)
            bq = const.tile([128, GP], F32)
            bk = const.tile([128, GP], F32)
            ones_col = const.tile([128, 1], BF16)
            e0_blk = const.tile([128, D], BF16)
            e32_blk = const.tile([128, D], BF16)
            v64 = const.tile([128, KT, 8, D], BF16)  # this core's 8 heads
            ao = const.tile([128, NPAIR, N], BF16)
            pav_sb = const.tile([128, NPASS, 512], BF16)
            rcp_sb = const.tile([33, NPASS, 512], BF16)
            # proj partial staging: pairs 0-2 contribution per (qc, od),
            # [128 od-dims, qc, od, 512 queries]
            prt_sb = const.tile([128, NQC, CT, 512], BF16)

            nc.vector.memset(ones_col[:], 1.0)
            nc.vector.memset(e0_blk[:], 0.0)
            nc.vector.memset(e32_blk[:], 0.0)
            nc.vector.memset(e0_blk[0:1, :], 1.0)
            nc.vector.memset(e32_blk[32:33, :], 1.0)

            # DMA startup order == prologue consumption order, using few BIG
            # multi-kc descriptors (per-issue overhead on the sync queue is
            # ~650 ns; 32 small issues would gate the prologue). Priority:
            # prologue inputs (x 0-1023, pair-0 K/Q weights, V weights),
            # then x 1024-2047 (vb needs it by kt 8 of pass 0), then the
            # remaining weights.
            nc.sync.dma_start(out=xT[:, :, 0:512], in_=xT_r[:, :, 0:512])
            nc.sync.dma_start(out=wk[:, :, 0:128], in_=wk_r[:, :, 0:128])
            nc.sync.dma_start(out=wv[:, :, :], in_=wv_r[:, :, :])
            nc.sync.dma_start(out=bk[:], in_=bk_ext[:].rearrange("(o p) 1 -> p o", p=128))
            nc.sync.dma_start(out=bq[:], in_=bq_ext[:].rearrange("(o p) 1 -> p o", p=128))
            nc.sync.dma_start(out=wq[:, :, 0:128], in_=wq_r[:, :, 0:128])
            nc.sync.dma_start(out=xT[:, :, 512:1024], in_=xT_r[:, :, 512:1024])
            nc.sync.dma_start(out=xT[:, :, 1024:1536], in_=xT_r[:, :, 1024:1536])
            nc.sync.dma_start(out=xT[:, :, 1536:N], in_=xT_r[:, :, 1536:N])
            nc.sync.dma_start(out=wk[:, :, 128:G], in_=wk_r[:, :, 128:G])
            nc.sync.dma_start(out=wq[:, :, 128:G], in_=wq_r[:, :, 128:G])
            nc.sync.dma_start(out=wp[:, :, :], in_=wp_r[:, :, :])

            # dens psum rows 1-31 are read by the batched reciprocal but never
            # written by the M=1 denominator matmuls; preset once to 1.0.
            dens_init = ps_den.tile([128, 512], F32, name="dens", tag="dens")
            nc.vector.memset(dens_init[0:33, :], 1.0)

            # ---- background work: fine-grained chunk generators ----------
            def gen_k(mt, dst, t0, t1):
                """K projection for pair mt, token chunks [t0, t1)."""
                for t in range(t0, t1):
                    p = ps_bg.tile([128, 512], F32, tag="bg")
                    for kc in range(CT):
                        nc.tensor.matmul(
                            p[:],
                            lhsT=wk[:, kc, mt * 128 : (mt + 1) * 128],
                            rhs=xT[:, kc, t * 512 : (t + 1) * 512],
                            start=(kc == 0),
                            stop=(kc == CT - 1),
                            skip_group_check=True,
                        )
                        yield
                    nc.vector.tensor_tensor(
                        dst[:, t * 512 : (t + 1) * 512],
                        p[:],
                        bk[:, mt : mt + 1].to_broadcast([128, 512]),
                        mybir.AluOpType.add,
                    )

            def gen_q(mt, dst, t0, t1):
                """Q projection for pair mt, query chunks [t0, t1)."""
                for t in range(t0, t1):
                    p = ps_bg.tile([128, 512], F32, tag="bg")
                    for kc in range(CT):
                        nc.tensor.matmul(
                            p[:],
                            lhsT=wq[:, kc, mt * 128 : (mt + 1) * 128],
                            rhs=xT[:, kc, t * 512 : (t + 1) * 512],
                            start=(kc == 0),
                            stop=(kc == CT - 1),
                            skip_group_check=True,
                        )
                        yield
                    nc.vector.tensor_tensor(
                        dst[:, t * 512 : (t + 1) * 512],
                        p[:],
                        bq[:, mt : mt + 1].to_broadcast([128, 512]),
                        mybir.AluOpType.add,
                    )

            def gen_v(tt0, tt1):
                """V projection (all 8 heads), key tiles [tt0, tt1)."""
                for tt in range(tt0, tt1):
                    p = ps_bg.tile([128, 512], F32, tag="bg")
                    for kc in range(CT):
                        nc.tensor.matmul(
                            p[:],
                            lhsT=xT[:, kc, tt * 128 : (tt + 1) * 128],
                            rhs=wv[:, kc, :],
                            start=(kc == 0),
                            stop=(kc == CT - 1),
                            skip_group_check=True,
                        )
                        yield
                    nc.vector.tensor_copy(
                        v64[:, tt, :, :],
                        p[:].rearrange("p (h d) -> p h d", d=D),
                    )

            def gen_norm(ps):
                """Normalize pass ps=(mt, qc): broadcast 1/den, scale pav -> ao."""
                mt, qc = ps // NQC, ps % NQC
                pbc = ps_bg.tile([128, 512], F32, tag="bg")
                nc.tensor.matmul(
                    pbc[0:D, :], lhsT=e0_blk[0:33, :], rhs=rcp_sb[:, ps, :],
                    start=True, stop=True, skip_group_check=True,
                )
                yield
                nc.tensor.matmul(
                    pbc[D:128, :], lhsT=e32_blk[0:33, :], rhs=rcp_sb[:, ps, :],
                    start=True, stop=True,
                    tile_position=(0, D), skip_group_check=True,
                )
                yield
                nc.vector.tensor_tensor(
                    ao[:, mt, qc * 512 : (qc + 1) * 512],
                    pbc[:],
                    pav_sb[:, ps, :],
                    mybir.AluOpType.mult,
                )

            def gen_proj_part(qc):
                """Pairs 0-2 of the output projection for query chunk qc,
                staged to SBUF bf16 (needs n(qc), n(4+qc), n(8+qc))."""
                for od in range(CT):
                    p = ps_bg.tile([128, 512], F32, tag="bg")
                    for mt in range(NPAIR - 1):
                        nc.tensor.matmul(
                            p[:],
                            lhsT=wp[:, mt, od * 128 : (od + 1) * 128],
                            rhs=ao[:, mt, qc * 512 : (qc + 1) * 512],
                            start=(mt == 0),
                            stop=(mt == NPAIR - 2),
                            skip_group_check=True,
                        )
                        yield
                    nc.vector.tensor_copy(prt_sb[:, qc, od, :], p[:])

            def gen_proj_fin(qc, wide=False):
                """Last pair (mt=3) + staged partial -> out (needs n(12+qc)).

                wide=True (tail only): od-pairs through the (then-idle)
                scores psum pool — halves the MM->TT->DMA sem hops that
                pace the tail's shallow pipeline."""
                if wide:
                    for od in range(0, CT, 2):
                        p = ps_s.tile([128, 2, 512], F32, tag="pss")
                        for j in range(2):
                            nc.tensor.matmul(
                                p[:, j, :],
                                lhsT=wp[:, NPAIR - 1, (od + j) * 128 : (od + j + 1) * 128],
                                rhs=ao[:, NPAIR - 1, qc * 512 : (qc + 1) * 512],
                                start=True, stop=True,
                                skip_group_check=True,
                            )
                            yield
                        o_st = ostp.tile([128, 2, 512], BF16, tag="ost2")
                        nc.vector.tensor_tensor(
                            o_st[:],
                            p[:],
                            prt_sb[:, qc, od : od + 2, :],
                            mybir.AluOpType.add,
                        )
                        nc.sync.dma_start(
                            out=out_r[:, od : od + 2, qc * 512 : (qc + 1) * 512],
                            in_=o_st[:],
                        )
                    return
                for od in range(CT):
                    p = ps_bg.tile([128, 512], F32, tag="bg")
                    nc.tensor.matmul(
                        p[:],
                        lhsT=wp[:, NPAIR - 1, od * 128 : (od + 1) * 128],
                        rhs=ao[:, NPAIR - 1, qc * 512 : (qc + 1) * 512],
                        start=True, stop=True,
                        skip_group_check=True,
                    )
                    yield
                    o_st = ostp.tile([128, 512], BF16, tag="ost")
                    nc.vector.tensor_tensor(
                        o_st[:],
                        p[:],
                        prt_sb[:, qc, od, :],
                        mybir.AluOpType.add,
                    )
                    nc.sync.dma_start(
                        out=out_r[:, od, qc * 512 : (qc + 1) * 512], in_=o_st[:]
                    )

            # background queue machinery
            bg_queue = []
            bg_done = set()

            def bg_pump(n):
                done = 0
                while done < n and bg_queue:
                    try:
                        next(bg_queue[0][1])
                        done += 1
                    except StopIteration:
                        bg_done.add(bg_queue.pop(0)[0])

            def bg_require(*names):
                # pump MINIMALLY: an overshooting pump floods the in-order
                # PE queue with background matmuls ahead of the exp-critical
                # scores that follow this require in program order.
                while bg_queue and not all(n in bg_done for n in names):
                    bg_pump(1)

            def bg_drain():
                while bg_queue:
                    bg_pump(1 << 30)

            kq_tiles = {}

            def enqueue_pair(mt):
                kTn = kqp.tile([128, N], BF16, tag="kT")
                qTn = kqp.tile([128, N], BF16, tag="qT")
                kq_tiles[mt] = (kTn, qTn)
                bg_queue.append((f"k{mt}a", gen_k(mt, kTn, 0, 2)))
                bg_queue.append((f"q{mt}0", gen_q(mt, qTn, 0, 1)))
                bg_queue.append((f"k{mt}b", gen_k(mt, kTn, 2, 4)))
                bg_queue.append((f"q{mt}1", gen_q(mt, qTn, 1, 2)))
                bg_queue.append((f"q{mt}2", gen_q(mt, qTn, 2, 3)))
                bg_queue.append((f"q{mt}3", gen_q(mt, qTn, 3, 4)))

            # ---- prologue: order matches DMA arrival ---------------------
            # Only V key-tiles 0-3 are required up front; the rest of V and
            # K0's second half stream in as pass 0 consumes them.
            kT0 = kqp.tile([128, N], BF16, tag="kT")
            qT0 = kqp.tile([128, N], BF16, tag="qT")
            kq_tiles[0] = (kT0, qT0)
            bg_queue.append(("k0a0", gen_k(0, kT0, 0, 1)))
            bg_queue.append(("q00", gen_q(0, qT0, 0, 1)))
            bg_queue.append(("va0", gen_v(0, 4)))
            bg_require("k0a0", "q00", "va0")
            # consumption order == FIFO order == DMA arrival order
            bg_queue.append(("k0a1", gen_k(0, kT0, 1, 2)))
            bg_queue.append(("va1", gen_v(4, 8)))
            bg_queue.append(("k0b", gen_k(0, kT0, 2, 4)))
            bg_queue.append(("vb0", gen_v(8, 12)))
            bg_queue.append(("vb1", gen_v(12, KT)))
            bg_queue.append(("q01", gen_q(0, qT0, 1, 2)))
            bg_queue.append(("q02", gen_q(0, qT0, 2, 3)))
            bg_queue.append(("q03", gen_q(0, qT0, 3, 4)))

            # ---- attention ----------------------------------------------
            # 3/step spreads the ~560 background slots nearly evenly across
            # passes (~24/pass), keeping per-pass PE work under the ACT exp
            # floor; 5/step drains the queue in the first two passes of each
            # pair-block, making those passes PE-bound (~22us vs 16.7us).
            BG_PER_KT = 3

            # state carried across the hoisted pass boundary
            pending = {}  # (ps) -> {kt: at_tile} for hoisted scores

            def make_scores_exp(kTp, qTp, qc, at_tiles):
                def scores_exp(kt):
                    pss = ps_s.tile([128, 2, 512], F32, name="pss", tag="pss")
                    nc.tensor.matmul(
                        pss[:, 0, :],
                        lhsT=kTp[0:D, kt * 128 : (kt + 1) * 128],
                        rhs=qTp[0:D, qc * 512 : (qc + 1) * 512],
                        start=True, stop=True, skip_group_check=True,
                    )
                    nc.tensor.matmul(
                        pss[:, 1, :],
                        lhsT=kTp[D:128, kt * 128 : (kt + 1) * 128],
                        rhs=qTp[D:128, qc * 512 : (qc + 1) * 512],
                        start=True, stop=True, skip_group_check=True,
                    )
                    at = atp.tile([128, 2, 512], BF16, tag="at")
                    nc.scalar.activation(
                        at[:], pss[:],
                        mybir.ActivationFunctionType.Exp, scale=float(SCALE),
                    )
                    at_tiles[kt] = at

                return scores_exp

            for mt in range(NPAIR):
                if mt + 1 < NPAIR:
                    enqueue_pair(mt + 1)
                kTp, qTp = kq_tiles[mt]
                hl = 2 * mt
                for qc in range(NQC):
                    ps = NQC * mt + qc
                    if ps == 0:
                        bg_require("k0a0", "q00", "va0")
                    elif qc == 0:
                        bg_require(f"k{mt}a", f"q{mt}0", "vb1")
                    else:
                        bg_require(f"q{mt}{qc}")
                    pav = ps_av.tile([128, 512], F32, name="pav", tag="pav")
                    dens = ps_den.tile([128, 512], F32, name="dens", tag="dens")

                    at_tiles = pending.pop(ps, {})
                    scores_exp = make_scores_exp(kTp, qTp, qc, at_tiles)

                    def av_dens(kt):
                        at = at_tiles[kt]
                        first, last = kt == 0, kt == KT - 1
                        nc.tensor.matmul(
                            pav[0:D, :],
                            lhsT=v64[:, kt, hl, :],
                            rhs=at[:, 0, :],
                            start=first, stop=last,
                            skip_group_check=True,
                        )
                        nc.tensor.matmul(
                            pav[D:128, :],
                            lhsT=v64[:, kt, hl + 1, :],
                            rhs=at[:, 1, :],
                            start=first, stop=last,
                            tile_position=(0, D),
                            skip_group_check=True,
                        )

                    def dens_mm(kt):
                        at = at_tiles.pop(kt)
                        first, last = kt == 0, kt == KT - 1
                        nc.tensor.matmul(
                            dens[0:1, :],
                            lhsT=ones_col[:],
                            rhs=at[:, 0, :],
                            start=first, stop=last,
                            skip_group_check=True,
                        )
                        nc.tensor.matmul(
                            dens[32:33, :],
                            lhsT=ones_col[:],
                            rhs=at[:, 1, :],
                            start=first, stop=last,
                            tile_position=(0, 32),
                            skip_group_check=True,
                        )

                    if 0 not in at_tiles:
                        scores_exp(0)
                        scores_exp(1)
                    # ps 0: absorb the V/K prologue flood. ps 13-15: drain
                    # the projection finishers in-pass (a tail chunk costs
                    # ~3us serialized vs ~0.2us overlapped); passes 10-12
                    # stay at 3 — their proj-partial chunks are DVE-copy
                    # heavy and a faster pump there re-creates WAR stalls.
                    bg_per_kt = 12 if ps == 0 else (7 if ps >= 13 else BG_PER_KT)
                    for kt2 in range(0, KT, 2):
                        if ps == 0:
                            # progressive K/V availability for the first pass
                            if kt2 == 2:
                                bg_require("k0a1")
                            elif kt2 == 4:
                                bg_require("va1")
                            elif kt2 == 8:
                                bg_require("vb0")
                            elif kt2 == 12:
                                bg_require("vb1")
                        if kt2 + 2 < KT:
                            if kt2 + 2 == 8 and qc == 0:
                                bg_require(f"k{mt}b")
                            scores_exp(kt2 + 2)
                            scores_exp(kt2 + 3)
                        elif ps + 1 < NPASS:
                            # hoist next pass's first two scores+exp
                            nmt, nqc = (ps + 1) // NQC, (ps + 1) % NQC
                            if nqc == 0:
                                bg_require(f"k{nmt}a", f"q{nmt}0")
                            else:
                                bg_require(f"q{nmt}{nqc}")
                            nkT, nqT = kq_tiles[nmt]
                            nat = {}
                            nse = make_scores_exp(nkT, nqT, nqc, nat)
                            nse(0)
                            nse(1)
                            pending[ps + 1] = nat
                        # attention first, bg last within each step: a bg
                        # matmul stalled on a psum WAR (DVE drain) must not
                        # sit ahead of the exp-critical attention matmuls in
                        # the in-order PE queue. av before dens: dens(0)
                        # waits on the den-bank copy at a pass boundary.
                        av_dens(kt2)
                        av_dens(kt2 + 1)
                        dens_mm(kt2)
                        dens_mm(kt2 + 1)
                        bg_pump(bg_per_kt)
                    if qc == NQC - 1:
                        kq_tiles.pop(mt)
                    # ---- pass end: stash pav/den on DVE ------------------
                    # copy_pav first (av(0) of the next pass WARs on it;
                    # dens(0) comes later in the PE queue), copy_den next,
                    # and the slow 3.3us reciprocal of pass ps-1 LAST — it
                    # is only consumed by n(ps-1) a full pass later, and
                    # must never sit ahead of the copies in the DVE queue.
                    nc.vector.tensor_copy(pav_sb[:, ps, :], pav[:])
                    nc.vector.tensor_copy(rcp_sb[:, ps, :], dens[0:33, :])
                    if 1 <= ps <= 14:
                        with nc.allow_low_precision("softmax denominator bf16"):
                            nc.vector.reciprocal(
                                rcp_sb[:, ps - 1, :], rcp_sb[:, ps - 1, :]
                            )
                        bg_queue.append((f"n{ps - 1}", gen_norm(ps - 1)))
                        psn = ps - 1
                        if psn >= 8 and psn < 12:
                            # n(8+qc') emitted -> pairs 0-2 of chunk qc' ready
                            bg_queue.append((f"pp{psn - 8}", gen_proj_part(psn - 8)))
                        if psn >= 12:
                            bg_queue.append((f"pf{psn - 12}", gen_proj_fin(psn - 12)))
                    if ps == 14:
                        # endgame: reciprocal pass 14 immediately (not lagged)
                        # so n(14) and proj-fin(2) execute DURING pass 15
                        # instead of spilling into the tail.
                        with nc.allow_low_precision("softmax denominator bf16"):
                            nc.vector.reciprocal(rcp_sb[:, 14, :], rcp_sb[:, 14, :])
                        bg_queue.append(("n14", gen_norm(14)))
                        bg_queue.append(("pf2", gen_proj_fin(2)))

            # ---- tail: last normalize + last projection finisher ---------
            with nc.allow_low_precision("softmax denominator bf16"):
                nc.vector.reciprocal(
                    rcp_sb[:, NPASS - 1, :], rcp_sb[:, NPASS - 1, :]
                )
            bg_queue.append((f"n{NPASS - 1}", gen_norm(NPASS - 1)))
            bg_queue.append(("pf3", gen_proj_fin(3, wide=True)))
            bg_drain()

    _split_sync_waits(nc)
    return nc


_CACHED_NC = None


def _get_nc():
    global _CACHED_NC
    if _CACHED_NC is None:
        _CACHED_NC = build()
    return _CACHED_NC


def make_in_maps(x, w_qkv, b_qkv, w_proj, b_proj):
    bf = ml_dtypes.bfloat16
    xTs = []
    for b in range(B):
        xTs.append(np.ascontiguousarray(x[b].T.astype(bf)))  # [C, N]
    in_maps = []
    for core in range(NCORES):
        b = core // 2
        g = core % 2
        sl = slice(g * G, (g + 1) * G)
        in_maps.append(
            {
                "xT": xTs[b],
                "w_q": np.ascontiguousarray(w_qkv[:, sl].astype(bf)),
                "w_k": np.ascontiguousarray(w_qkv[:, C + g * G : C + (g + 1) * G].astype(bf)),
                "w_v": np.ascontiguousarray(w_qkv[:, 2 * C + g * G : 2 * C + (g + 1) * G].astype(bf)),
                "w_p": np.ascontiguousarray(w_proj[sl, :].astype(bf)),
                "b_q": np.ascontiguousarray(b_qkv[sl].reshape(G, 1).astype(np.float32)),
                "b_k": np.ascontiguousarray(
                    b_qkv[C + g * G : C + (g + 1) * G].reshape(G, 1).astype(np.float32)
                ),
            }
        )
    return in_maps


def run(x, w_qkv, b_qkv, w_proj, b_proj, trace=False, **spmd_kwargs):
    nc = _get_nc()
    in_maps = make_in_maps(x, w_qkv, b_qkv, w_proj, b_proj)
    res = run_bass_kernel_spmd(
        nc, in_maps, core_ids=list(range(NCORES)), trace=trace, **spmd_kwargs
    )
    # host-side reduction of the two head-half partials + folded bias
    b_v = b_qkv[2 * C : 3 * C].astype(np.float32)
    bias = b_proj.astype(np.float32) + b_v @ w_proj.astype(np.float32)  # [C]
    out = np.empty((B, N, C), dtype=np.float32)
    for b in range(B):
        yT = res.results[2 * b]["out"].astype(np.float32) + res.results[
            2 * b + 1
        ]["out"].astype(np.float32)  # [C, N]
        out[b] = yT.T + bias
    return out, res


def kernel(x, w_qkv, b_qkv, w_proj, b_proj):
    x = np.asarray(x, dtype=np.float32)
    w_qkv = np.asarray(w_qkv, dtype=np.float32)
    b_qkv = np.asarray(b_qkv, dtype=np.float32)
    w_proj = np.asarray(w_proj, dtype=np.float32)
    b_proj = np.asarray(b_proj, dtype=np.float32)
    out, _ = run(x, w_qkv, b_qkv, w_proj, b_proj, trace=False)
    return out


# revision 12
# speedup vs baseline: 1.0031x; 1.0031x over previous
"""Multi-head attention v3.1 on 8 TRN2 NeuronCores.

Core c = (batch b = c//2, head-half g = c%2): heads [8g, 8g+8) over ALL
2048 queries. Output projection is computed as a PARTIAL product
ao_g @ w_proj[512 g-rows] on each core; the host sums the two partials
per batch and adds the (folded) bias — zero collectives.

vs v2 ((batch, query-half) split): K and V projections are no longer
duplicated across the two cores of a batch (bg matmul work drops from
768 to 512 slots/core) and weight DMA halves.

Schedule (v3.1):
 - pass = (head-pair mt, query-chunk qc), 16 passes; ScalarE exp
   ([128,1024]-free ~1147 ns) paces the inner loop;
 - the next pass's first two scores+exp are HOISTED into the current
   pass's last kt2 step so the ACT queue never drains at a boundary;
 - pass-end pav/den stashes run on DVE (ACT stays pure-exp);
 - output projection is split per query-chunk into a 3-chunk partial
   (staged to SBUF as soon as pairs 0-2 are normalized) + 1-chunk
   finisher, so only proj-fin(qc=3) (8 matmuls) remains after the last
   pass;
 - prologue FIFO order matches the DMA arrival order (x tokens 0-1023
   first) so the PE never waits on late DMA.
"""

import sys

if "/opt/trn_rl_repo" not in sys.path:
    sys.path.insert(0, "/opt/trn_rl_repo")

import numpy as np
import ml_dtypes

import concourse.bass as bass
import concourse.mybir as mybir
from concourse.tile import TileContext
from concourse.bass_utils import run_bass_kernel_spmd

F32 = mybir.dt.float32
BF16 = mybir.dt.bfloat16

B = 4
N = 2048
C = 1024
H = 16
D = 64
SCALE = D**-0.5
NCORES = 8
CT = C // 128  # 8 contraction tiles over C
KT = N // 128  # 16 key tiles
G = 512  # dims per head-group (8 heads x 64)
GP = G // 128  # 4 partition tiles over the group dims
NPAIR = 4  # head pairs per core
NQC = 4  # query chunks of 512
NPASS = NPAIR * NQC  # 16 passes


def _split_sync_waits(nc, max_waits: int = 1) -> int:
    """Walrus rejects TPB instructions with >1 sync-wait; hoist extras onto
    InstNoOps inserted just before, on the same engine."""
    n_split = 0
    for fn in nc.m.functions:
        for block in fn.blocks:
            out = []
            changed = False
            for inst in block.instructions:
                si = getattr(inst, "sync_info", None)
                if si is not None and len(si.on_wait) > max_waits:
                    waits = list(si.on_wait)
                    n_extra = len(waits) - max_waits
                    for i in range(0, n_extra, max_waits):
                        out.append(
                            mybir.InstNoOp(
                                name=f"{inst.name}-sw{i}",
                                sync_info=mybir.SyncInfo(
                                    on_wait=waits[i : i + max_waits], on_update=[]
                                ),
                                bass_nofuse=True,
                                engine=inst.engine,
                            )
                        )
                    inst.sync_info = mybir.SyncInfo(
                        on_wait=waits[n_extra:], on_update=list(si.on_update)
                    )
                    changed = True
                    n_split += 1
                out.append(inst)
            if changed:
                block.instructions = out
    return n_split


def build():
    nc = bass.Bass(target_bir_lowering=False)

    xT_ext = nc.declare_dram_parameter("xT", [C, N], BF16, isOutput=False)
    wq_ext = nc.declare_dram_parameter("w_q", [C, G], BF16, isOutput=False)
    wk_ext = nc.declare_dram_parameter("w_k", [C, G], BF16, isOutput=False)
    wv_ext = nc.declare_dram_parameter("w_v", [C, G], BF16, isOutput=False)
    wp_ext = nc.declare_dram_parameter("w_p", [G, C], BF16, isOutput=False)
    bq_ext = nc.declare_dram_parameter("b_q", [G, 1], F32, isOutput=False)
    bk_ext = nc.declare_dram_parameter("b_k", [G, 1], F32, isOutput=False)
    out_ext = nc.declare_dram_parameter("out", [C, N], BF16, isOutput=True)

    xT_r = xT_ext[:].rearrange("(o p) n -> p o n", p=128)
    wq_r = wq_ext[:].rearrange("(o p) n -> p o n", p=128)
    wk_r = wk_ext[:].rearrange("(o p) n -> p o n", p=128)
    wv_r = wv_ext[:].rearrange("(o p) n -> p o n", p=128)
    wp_r = wp_ext[:].rearrange("(o p) n -> p o n", p=128)
    out_r = out_ext[:].rearrange("(o p) n -> p o n", p=128)

    with TileContext(nc) as tc:
        with (
            tc.tile_pool(name="const", bufs=1) as const,
            tc.tile_pool(name="kq", bufs=2) as kqp,
            tc.tile_pool(name="at", bufs=4) as atp,
            tc.tile_pool(name="ost", bufs=2) as ostp,
            tc.tile_pool(name="ps_s", bufs=2, space="PSUM") as ps_s,
            tc.tile_pool(name="ps_av", bufs=1, space="PSUM") as ps_av,
            tc.tile_pool(name="ps_den", bufs=1, space="PSUM") as ps_den,
            tc.tile_pool(name="ps_bg", bufs=2, space="PSUM") as ps_bg,
        ):
            # ---- constants / big residents -------------------------------
            xT = const.tile([128, CT, N], BF16)
            wq = const.tile([128, CT, G], BF16)
            wk = const.tile([128, CT, G], BF16)
            wv = const.tile([128, CT, G], BF16)
            wp = const.tile([128, GP, C], BF16)
            bq = const.tile([128, GP], F32)
            bk = const.tile([128, GP], F32)
            ones_col = const.tile([128, 1], BF16)
            e0_blk = const.tile([128, D], BF16)
            e32_blk = const.tile([128, D], BF16)
            v64 = const.tile([128, KT, 8, D], BF16)  # this core's 8 heads
            ao = const.tile([128, NPAIR, N], BF16)
            pav_sb = const.tile([128, NPASS, 512], BF16)
            rcp_sb = const.tile([33, NPASS, 512], BF16)
            # proj partial staging: pairs 0-2 contribution per (qc, od),
            # [128 od-dims, qc, od, 512 queries]
            prt_sb = const.tile([128, NQC, CT, 512], BF16)

            nc.vector.memset(ones_col[:], 1.0)
            nc.vector.memset(e0_blk[:], 0.0)
            nc.vector.memset(e32_blk[:], 0.0)
            nc.vector.memset(e0_blk[0:1, :], 1.0)
            nc.vector.memset(e32_blk[32:33, :], 1.0)

            # DMA startup order == prologue consumption order, using few BIG
            # multi-kc descriptors (per-issue overhead on the sync queue is
            # ~650 ns; 32 small issues would gate the prologue). Priority:
            # prologue inputs (x 0-1023, pair-0 K/Q weights, V weights),
            # then x 1024-2047 (vb needs it by kt 8 of pass 0), then the
            # remaining weights.
            nc.sync.dma_start(out=xT[:, :, 0:512], in_=xT_r[:, :, 0:512])
            nc.sync.dma_start(out=wk[:, :, 0:128], in_=wk_r[:, :, 0:128])
            nc.sync.dma_start(out=wv[:, :, :], in_=wv_r[:, :, :])
            nc.sync.dma_start(out=bk[:], in_=bk_ext[:].rearrange("(o p) 1 -> p o", p=128))
            nc.sync.dma_start(out=bq[:], in_=bq_ext[:].rearrange("(o p) 1 -> p o", p=128))
            nc.sync.dma_start(out=wq[:, :, 0:128], in_=wq_r[:, :, 0:128])
            nc.sync.dma_start(out=xT[:, :, 512:1024], in_=xT_r[:, :, 512:1024])
            nc.sync.dma_start(out=xT[:, :, 1024:1536], in_=xT_r[:, :, 1024:1536])
            nc.sync.dma_start(out=xT[:, :, 1536:N], in_=xT_r[:, :, 1536:N])
            nc.sync.dma_start(out=wk[:, :, 128:G], in_=wk_r[:, :, 128:G])
            nc.sync.dma_start(out=wq[:, :, 128:G], in_=wq_r[:, :, 128:G])
            nc.sync.dma_start(out=wp[:, :, :], in_=wp_r[:, :, :])

            # dens psum rows 1-31 are read by the batched reciprocal but never
            # written by the M=1 denominator matmuls; preset once to 1.0.
            dens_init = ps_den.tile([128, 512], F32, name="dens", tag="dens")
            nc.vector.memset(dens_init[0:33, :], 1.0)

            # ---- background work: fine-grained chunk generators ----------
            def gen_k(mt, dst, t0, t1):
                """K projection for pair mt, token chunks [t0, t1)."""
                for t in range(t0, t1):
                    p = ps_bg.tile([128, 512], F32, tag="bg")
                    for kc in range(CT):
                        nc.tensor.matmul(
                            p[:],
                            lhsT=wk[:, kc, mt * 128 : (mt + 1) * 128],
                            rhs=xT[:, kc, t * 512 : (t + 1) * 512],
                            start=(kc == 0),
                            stop=(kc == CT - 1),
                            skip_group_check=True,
                        )
                        yield
                    nc.vector.tensor_tensor(
                        dst[:, t * 512 : (t + 1) * 512],
                        p[:],
                        bk[:, mt : mt + 1].to_broadcast([128, 512]),
                        mybir.AluOpType.add,
                    )

            def gen_q(mt, dst, t0, t1):
                """Q projection for pair mt, query chunks [t0, t1)."""
                for t in range(t0, t1):
                    p = ps_bg.tile([128, 512], F32, tag="bg")
                    for kc in range(CT):
                        nc.tensor.matmul(
                            p[:],
                            lhsT=wq[:, kc, mt * 128 : (mt + 1) * 128],
                            rhs=xT[:, kc, t * 512 : (t + 1) * 512],
                            start=(kc == 0),
                            stop=(kc == CT - 1),
                            skip_group_check=True,
                        )
                        yield
                    nc.vector.tensor_tensor(
                        dst[:, t * 512 : (t + 1) * 512],
                        p[:],
                        bq[:, mt : mt + 1].to_broadcast([128, 512]),
                        mybir.AluOpType.add,
                    )

            def gen_v(tt0, tt1):
                """V projection (all 8 heads), key tiles [tt0, tt1)."""
                for tt in range(tt0, tt1):
                    p = ps_bg.tile([128, 512], F32, tag="bg")
                    for kc in range(CT):
                        nc.tensor.matmul(
                            p[:],
                            lhsT=xT[:, kc, tt * 128 : (tt + 1) * 128],
                            rhs=wv[:, kc, :],
                            start=(kc == 0),
                            stop=(kc == CT - 1),
                            skip_group_check=True,
                        )
                        yield
                    nc.vector.tensor_copy(
                        v64[:, tt, :, :],
                        p[:].rearrange("p (h d) -> p h d", d=D),
                    )

            def gen_norm(ps):
                """Normalize pass ps=(mt, qc): broadcast 1/den, scale pav -> ao."""
                mt, qc = ps // NQC, ps % NQC
                pbc = ps_bg.tile([128, 512], F32, tag="bg")
                nc.tensor.matmul(
                    pbc[0:D, :], lhsT=e0_blk[0:33, :], rhs=rcp_sb[:, ps, :],
                    start=True, stop=True, skip_group_check=True,
                )
                yield
                nc.tensor.matmul(
                    pbc[D:128, :], lhsT=e32_blk[0:33, :], rhs=rcp_sb[:, ps, :],
                    start=True, stop=True,
                    tile_position=(0, D), skip_group_check=True,
                )
                yield
                nc.vector.tensor_tensor(
                    ao[:, mt, qc * 512 : (qc + 1) * 512],
                    pbc[:],
                    pav_sb[:, ps, :],
                    mybir.AluOpType.mult,
                )

            def gen_proj_part(qc):
                """Pairs 0-2 of the output projection for query chunk qc,
                staged to SBUF bf16 (needs n(qc), n(4+qc), n(8+qc))."""
                for od in range(CT):
                    p = ps_bg.tile([128, 512], F32, tag="bg")
                    for mt in range(NPAIR - 1):
                        nc.tensor.matmul(
                            p[:],
                            lhsT=wp[:, mt, od * 128 : (od + 1) * 128],
                            rhs=ao[:, mt, qc * 512 : (qc + 1) * 512],
                            start=(mt == 0),
                            stop=(mt == NPAIR - 2),
                            skip_group_check=True,
                        )
                        yield
                    nc.vector.tensor_copy(prt_sb[:, qc, od, :], p[:])

            def gen_proj_fin(qc, wide=False):
                """Last pair (mt=3) + staged partial -> out (needs n(12+qc)).

                wide=True (tail only): od-pairs through the (then-idle)
                scores psum pool — halves the MM->TT->DMA sem hops that
                pace the tail's shallow pipeline."""
                if wide:
                    for od in range(0, CT, 2):
                        p = ps_s.tile([128, 2, 512], F32, tag="pss")
                        for j in range(2):
                            nc.tensor.matmul(
                                p[:, j, :],
                                lhsT=wp[:, NPAIR - 1, (od + j) * 128 : (od + j + 1) * 128],
                                rhs=ao[:, NPAIR - 1, qc * 512 : (qc + 1) * 512],
                                start=True, stop=True,
                                skip_group_check=True,
                            )
                            yield
                        o_st = ostp.tile([128, 2, 512], BF16, tag="ost2")
                        nc.vector.tensor_tensor(
                            o_st[:],
                            p[:],
                            prt_sb[:, qc, od : od + 2, :],
                            mybir.AluOpType.add,
                        )
                        nc.sync.dma_start(
                            out=out_r[:, od : od + 2, qc * 512 : (qc + 1) * 512],
                            in_=o_st[:],
                        )
                    return
                for od in range(CT):
                    p = ps_bg.tile([128, 512], F32, tag="bg")
                    nc.tensor.matmul(
                        p[:],
                        lhsT=wp[:, NPAIR - 1, od * 128 : (od + 1) * 128],
                        rhs=ao[:, NPAIR - 1, qc * 512 : (qc + 1) * 512],
                        start=True, stop=True,
                        skip_group_check=True,
                    )
                    yield
                    o_st = ostp.tile([128, 512], BF16, tag="ost")
                    nc.vector.tensor_tensor(
                        o_st[:],
                        p[:],
                        prt_sb[:, qc, od, :],
                        mybir.AluOpType.add,
                    )
                    nc.sync.dma_start(
                        out=out_r[:, od, qc * 512 : (qc + 1) * 512], in_=o_st[:]
                    )

            # background queue machinery
            bg_queue = []
            bg_done = set()

            def bg_pump(n):
                done = 0
                while done < n and bg_queue:
                    try:
                        next(bg_queue[0][1])
                        done += 1
                    except StopIteration:
                        bg_done.add(bg_queue.pop(0)[0])

            def bg_require(*names):
                # pump MINIMALLY: an overshooting pump floods the in-order
                # PE queue with background matmuls ahead of the exp-critical
                # scores that follow this require in program order.
                while bg_queue and not all(n in bg_done for n in names):
                    bg_pump(1)

            def bg_drain():
                while bg_queue:
                    bg_pump(1 << 30)

            kq_tiles = {}

            def enqueue_pair(mt):
                kTn = kqp.tile([128, N], BF16, tag="kT")
                qTn = kqp.tile([128, N], BF16, tag="qT")
                kq_tiles[mt] = (kTn, qTn)
                bg_queue.append((f"k{mt}a", gen_k(mt, kTn, 0, 2)))
                bg_queue.append((f"q{mt}0", gen_q(mt, qTn, 0, 1)))
                bg_queue.append((f"k{mt}b", gen_k(mt, kTn, 2, 4)))
                bg_queue.append((f"q{mt}1", gen_q(mt, qTn, 1, 2)))
                bg_queue.append((f"q{mt}2", gen_q(mt, qTn, 2, 3)))
                bg_queue.append((f"q{mt}3", gen_q(mt, qTn, 3, 4)))

            # ---- prologue: order matches DMA arrival ---------------------
            # Only V key-tiles 0-3 are required up front; the rest of V and
            # K0's second half stream in as pass 0 consumes them.
            kT0 = kqp.tile([128, N], BF16, tag="kT")
            qT0 = kqp.tile([128, N], BF16, tag="qT")
            kq_tiles[0] = (kT0, qT0)
            bg_queue.append(("k0a0", gen_k(0, kT0, 0, 1)))
            bg_queue.append(("q00", gen_q(0, qT0, 0, 1)))
            bg_queue.append(("va0", gen_v(0, 4)))
            bg_require("k0a0", "q00", "va0")
            # consumption order == FIFO order == DMA arrival order
            bg_queue.append(("k0a1", gen_k(0, kT0, 1, 2)))
            bg_queue.append(("va1", gen_v(4, 8)))
            bg_queue.append(("k0b", gen_k(0, kT0, 2, 4)))
            bg_queue.append(("vb0", gen_v(8, 12)))
            bg_queue.append(("vb1", gen_v(12, KT)))
            bg_queue.append(("q01", gen_q(0, qT0, 1, 2)))
            bg_queue.append(("q02", gen_q(0, qT0, 2, 3)))
            bg_queue.append(("q03", gen_q(0, qT0, 3, 4)))

            # ---- attention ----------------------------------------------
            # 3/step spreads the ~560 background slots nearly evenly across
            # passes (~24/pass), keeping per-pass PE work under the ACT exp
            # floor; 5/step drains the queue in the first two passes of each
            # pair-block, making those passes PE-bound (~22us vs 16.7us).
            BG_PER_KT = 3

            # state carried across the hoisted pass boundary
            pending = {}  # (ps) -> {kt: at_tile} for hoisted scores

            def make_scores_exp(kTp, qTp, qc, at_tiles):
                def scores_exp(kt):
                    pss = ps_s.tile([128, 2, 512], F32, name="pss", tag="pss")
                    nc.tensor.matmul(
                        pss[:, 0, :],
                        lhsT=kTp[0:D, kt * 128 : (kt + 1) * 128],
                        rhs=qTp[0:D, qc * 512 : (qc + 1) * 512],
                        start=True, stop=True, skip_group_check=True,
                    )
                    nc.tensor.matmul(
                        pss[:, 1, :],
                        lhsT=kTp[D:128, kt * 128 : (kt + 1) * 128],
                        rhs=qTp[D:128, qc * 512 : (qc + 1) * 512],
                        start=True, stop=True, skip_group_check=True,
                    )
                    at = atp.tile([128, 2, 512], BF16, tag="at")
                    nc.scalar.activation(
                        at[:], pss[:],
                        mybir.ActivationFunctionType.Exp, scale=float(SCALE),
                    )
                    at_tiles[kt] = at

                return scores_exp

            for mt in range(NPAIR):
                if mt + 1 < NPAIR:
                    enqueue_pair(mt + 1)
                kTp, qTp = kq_tiles[mt]
                hl = 2 * mt
                for qc in range(NQC):
                    ps = NQC * mt + qc
                    if ps == 0:
                        bg_require("k0a0", "q00", "va0")
                    elif qc == 0:
                        bg_require(f"k{mt}a", f"q{mt}0", "vb1")
                    else:
                        bg_require(f"q{mt}{qc}")
                    pav = ps_av.tile([128, 512], F32, name="pav", tag="pav")
                    dens = ps_den.tile([128, 512], F32, name="dens", tag="dens")

                    at_tiles = pending.pop(ps, {})
                    scores_exp = make_scores_exp(kTp, qTp, qc, at_tiles)

                    def av_dens(kt):
                        at = at_tiles[kt]
                        first, last = kt == 0, kt == KT - 1
                        nc.tensor.matmul(
                            pav[0:D, :],
                            lhsT=v64[:, kt, hl, :],
                            rhs=at[:, 0, :],
                            start=first, stop=last,
                            skip_group_check=True,
                        )
                        nc.tensor.matmul(
                            pav[D:128, :],
                            lhsT=v64[:, kt, hl + 1, :],
                            rhs=at[:, 1, :],
                            start=first, stop=last,
                            tile_position=(0, D),
                            skip_group_check=True,
                        )

                    def dens_mm(kt):
                        at = at_tiles.pop(kt)
                        first, last = kt == 0, kt == KT - 1
                        nc.tensor.matmul(
                            dens[0:1, :],
                            lhsT=ones_col[:],
                            rhs=at[:, 0, :],
                            start=first, stop=last,
                            skip_group_check=True,
                        )
                        nc.tensor.matmul(
                            dens[32:33, :],
                            lhsT=ones_col[:],
                            rhs=at[:, 1, :],
                            start=first, stop=last,
                            tile_position=(0, 32),
                            skip_group_check=True,
                        )

                    if 0 not in at_tiles:
                        scores_exp(0)
                        scores_exp(1)
                    # ps 0: absorb the V/K prologue flood. ps 13-15: drain
                    # the projection finishers in-pass (a tail chunk costs
                    # ~3us serialized vs ~0.2us overlapped); passes 10-12
                    # stay at 3 — their proj-partial chunks are DVE-copy
                    # heavy and a faster pump there re-creates WAR stalls.
                    bg_per_kt = 12 if ps == 0 else (5 if ps >= 13 else BG_PER_KT)
                    for kt2 in range(0, KT, 2):
                        if ps == 0 and kt2 == 2:
                            # scores(4/5) read kT tokens 512-1023
                            bg_require("k0a1")
                        if kt2 + 2 < KT:
                            if kt2 + 2 == 8 and qc == 0:
                                bg_require(f"k{mt}b")
                            scores_exp(kt2 + 2)
                            scores_exp(kt2 + 3)
                        elif ps + 1 < NPASS:
                            # hoist next pass's first two scores+exp
                            nmt, nqc = (ps + 1) // NQC, (ps + 1) % NQC
                            if nqc == 0:
                                bg_require(f"k{nmt}a", f"q{nmt}0")
                            else:
                                bg_require(f"q{nmt}{nqc}")
                            nkT, nqT = kq_tiles[nmt]
                            nat = {}
                            nse = make_scores_exp(nkT, nqT, nqc, nat)
                            nse(0)
                            nse(1)
                            pending[ps + 1] = nat
                        if ps == 0:
                            # progressive V availability: these requires only
                            # protect av_dens below — emitted AFTER scores so
                            # the inline V-generation flood cannot block the
                            # exp-critical scores in the in-order PE queue.
                            if kt2 == 4:
                                bg_require("va1")
                            elif kt2 == 8:
                                bg_require("vb0")
                            elif kt2 == 12:
                                bg_require("vb1")
                        # attention first, bg last within each step: a bg
                        # matmul stalled on a psum WAR (DVE drain) must not
                        # sit ahead of the exp-critical attention matmuls in
                        # the in-order PE queue. av before dens: dens(0)
                        # waits on the den-bank copy at a pass boundary.
                        av_dens(kt2)
                        av_dens(kt2 + 1)
                        dens_mm(kt2)
                        dens_mm(kt2 + 1)
                        bg_pump(bg_per_kt)
                    if qc == NQC - 1:
                        kq_tiles.pop(mt)
                    # ---- pass end: stash pav/den on DVE ------------------
                    # copy_pav first (av(0) of the next pass WARs on it;
                    # dens(0) comes later in the PE queue), copy_den next,
                    # and the slow 3.3us reciprocal of pass ps-1 LAST — it
                    # is only consumed by n(ps-1) a full pass later, and
                    # must never sit ahead of the copies in the DVE queue.
                    nc.vector.tensor_copy(pav_sb[:, ps, :], pav[:])
                    nc.vector.tensor_copy(rcp_sb[:, ps, :], dens[0:33, :])
                    if 1 <= ps <= 14:
                        with nc.allow_low_precision("softmax denominator bf16"):
                            nc.vector.reciprocal(
                                rcp_sb[:, ps - 1, :], rcp_sb[:, ps - 1, :]
                            )
                        bg_queue.append((f"n{ps - 1}", gen_norm(ps - 1)))
                        psn = ps - 1
                        if psn >= 8 and psn < 12:
                            # n(8+qc') emitted -> pairs 0-2 of chunk qc' ready
                            bg_queue.append((f"pp{psn - 8}", gen_proj_part(psn - 8)))
                        if psn >= 12:
                            bg_queue.append((f"pf{psn - 12}", gen_proj_fin(psn - 12)))
                    if ps == 14:
                        # endgame: reciprocal pass 14 immediately (not lagged)
                        # so n(14) and proj-fin(2) execute DURING pass 15
                        # instead of spilling into the tail.
                        with nc.allow_low_precision("softmax denominator bf16"):
                            nc.vector.reciprocal(rcp_sb[:, 14, :], rcp_sb[:, 14, :])
                        bg_queue.append(("n14", gen_norm(14)))
                        bg_queue.append(("pf2", gen_proj_fin(2)))

            # ---- tail: last normalize + last projection finisher ---------
            with nc.allow_low_precision("softmax denominator bf16"):
                nc.vector.reciprocal(
                    rcp_sb[:, NPASS - 1, :], rcp_sb[:, NPASS - 1, :]
                )
            bg_queue.append((f"n{NPASS - 1}", gen_norm(NPASS - 1)))
            bg_queue.append(("pf3", gen_proj_fin(3, wide=True)))
            bg_drain()

    _split_sync_waits(nc)
    return nc


_CACHED_NC = None


def _get_nc():
    global _CACHED_NC
    if _CACHED_NC is None:
        _CACHED_NC = build()
    return _CACHED_NC


def make_in_maps(x, w_qkv, b_qkv, w_proj, b_proj):
    bf = ml_dtypes.bfloat16
    xTs = []
    for b in range(B):
        xTs.append(np.ascontiguousarray(x[b].T.astype(bf)))  # [C, N]
    in_maps = []
    for core in range(NCORES):
        b = core // 2
        g = core % 2
        sl = slice(g * G, (g + 1) * G)
        in_maps.append(
            {
                "xT": xTs[b],
                "w_q": np.ascontiguousarray(w_qkv[:, sl].astype(bf)),
                "w_k": np.ascontiguousarray(w_qkv[:, C + g * G : C + (g + 1) * G].astype(bf)),
                "w_v": np.ascontiguousarray(w_qkv[:, 2 * C + g * G : 2 * C + (g + 1) * G].astype(bf)),
                "w_p": np.ascontiguousarray(w_proj[sl, :].astype(bf)),
                "b_q": np.ascontiguousarray(b_qkv[sl].reshape(G, 1).astype(np.float32)),
                "b_k": np.ascontiguousarray(
                    b_qkv[C + g * G : C + (g + 1) * G].reshape(G, 1).astype(np.float32)
                ),
            }
        )
    return in_maps


def run(x, w_qkv, b_qkv, w_proj, b_proj, trace=False, **spmd_kwargs):
    nc = _get_nc()
    in_maps = make_in_maps(x, w_qkv, b_qkv, w_proj, b_proj)
    res = run_bass_kernel_spmd(
        nc, in_maps, core_ids=list(range(NCORES)), trace=trace, **spmd_kwargs
    )
    # host-side reduction of the two head-half partials + folded bias
    b_v = b_qkv[2 * C : 3 * C].astype(np.float32)
    bias = b_proj.astype(np.float32) + b_v @ w_proj.astype(np.float32)  # [C]
    out = np.empty((B, N, C), dtype=np.float32)
    for b in range(B):
        yT = res.results[2 * b]["out"].astype(np.float32) + res.results[
            2 * b + 1
        ]["out"].astype(np.float32)  # [C, N]
        out[b] = yT.T + bias
    return out, res


def kernel(x, w_qkv, b_qkv, w_proj, b_proj):
    x = np.asarray(x, dtype=np.float32)
    w_qkv = np.asarray(w_qkv, dtype=np.float32)
    b_qkv = np.asarray(b_qkv, dtype=np.float32)
    w_proj = np.asarray(w_proj, dtype=np.float32)
    b_proj = np.asarray(b_proj, dtype=np.float32)
    out, _ = run(x, w_qkv, b_qkv, w_proj, b_proj, trace=False)
    return out


# revision 13
# speedup vs baseline: 1.0183x; 1.0151x over previous
"""Multi-head attention v3.1 on 8 TRN2 NeuronCores.

Core c = (batch b = c//2, head-half g = c%2): heads [8g, 8g+8) over ALL
2048 queries. Output projection is computed as a PARTIAL product
ao_g @ w_proj[512 g-rows] on each core; the host sums the two partials
per batch and adds the (folded) bias — zero collectives.

vs v2 ((batch, query-half) split): K and V projections are no longer
duplicated across the two cores of a batch (bg matmul work drops from
768 to 512 slots/core) and weight DMA halves.

Schedule (v3.1):
 - pass = (head-pair mt, query-chunk qc), 16 passes; ScalarE exp
   ([128,1024]-free ~1147 ns) paces the inner loop;
 - the next pass's first two scores+exp are HOISTED into the current
   pass's last kt2 step so the ACT queue never drains at a boundary;
 - pass-end pav/den stashes run on DVE (ACT stays pure-exp);
 - output projection is split per query-chunk into a 3-chunk partial
   (staged to SBUF as soon as pairs 0-2 are normalized) + 1-chunk
   finisher, so only proj-fin(qc=3) (8 matmuls) remains after the last
   pass;
 - prologue FIFO order matches the DMA arrival order (x tokens 0-1023
   first) so the PE never waits on late DMA.
"""

import sys

if "/opt/trn_rl_repo" not in sys.path:
    sys.path.insert(0, "/opt/trn_rl_repo")

import numpy as np
import ml_dtypes

import concourse.bass as bass
import concourse.mybir as mybir
from concourse.tile import TileContext
from concourse.bass_utils import run_bass_kernel_spmd

F32 = mybir.dt.float32
BF16 = mybir.dt.bfloat16

B = 4
N = 2048
C = 1024
H = 16
D = 64
SCALE = D**-0.5
NCORES = 8
CT = C // 128  # 8 contraction tiles over C
KT = N // 128  # 16 key tiles
G = 512  # dims per head-group (8 heads x 64)
GP = G // 128  # 4 partition tiles over the group dims
NPAIR = 4  # head pairs per core
NQC = 4  # query chunks of 512
NPASS = NPAIR * NQC  # 16 passes


def _split_sync_waits(nc, max_waits: int = 1) -> int:
    """Walrus rejects TPB instructions with >1 sync-wait; hoist extras onto
    InstNoOps inserted just before, on the same engine."""
    n_split = 0
    for fn in nc.m.functions:
        for block in fn.blocks:
            out = []
            changed = False
            for inst in block.instructions:
                si = getattr(inst, "sync_info", None)
                if si is not None and len(si.on_wait) > max_waits:
                    waits = list(si.on_wait)
                    n_extra = len(waits) - max_waits
                    for i in range(0, n_extra, max_waits):
                        out.append(
                            mybir.InstNoOp(
                                name=f"{inst.name}-sw{i}",
                                sync_info=mybir.SyncInfo(
                                    on_wait=waits[i : i + max_waits], on_update=[]
                                ),
                                bass_nofuse=True,
                                engine=inst.engine,
                            )
                        )
                    inst.sync_info = mybir.SyncInfo(
                        on_wait=waits[n_extra:], on_update=list(si.on_update)
                    )
                    changed = True
                    n_split += 1
                out.append(inst)
            if changed:
                block.instructions = out
    return n_split


def build():
    nc = bass.Bass(target_bir_lowering=False)

    xT_ext = nc.declare_dram_parameter("xT", [C, N], BF16, isOutput=False)
    wq_ext = nc.declare_dram_parameter("w_q", [C, G], BF16, isOutput=False)
    wk_ext = nc.declare_dram_parameter("w_k", [C, G], BF16, isOutput=False)
    wv_ext = nc.declare_dram_parameter("w_v", [C, G], BF16, isOutput=False)
    wp_ext = nc.declare_dram_parameter("w_p", [G, C], BF16, isOutput=False)
    bq_ext = nc.declare_dram_parameter("b_q", [G, 1], F32, isOutput=False)
    bk_ext = nc.declare_dram_parameter("b_k", [G, 1], F32, isOutput=False)
    out_ext = nc.declare_dram_parameter("out", [C, N], BF16, isOutput=True)

    xT_r = xT_ext[:].rearrange("(o p) n -> p o n", p=128)
    wq_r = wq_ext[:].rearrange("(o p) n -> p o n", p=128)
    wk_r = wk_ext[:].rearrange("(o p) n -> p o n", p=128)
    wv_r = wv_ext[:].rearrange("(o p) n -> p o n", p=128)
    wp_r = wp_ext[:].rearrange("(o p) n -> p o n", p=128)
    out_r = out_ext[:].rearrange("(o p) n -> p o n", p=128)

    with TileContext(nc) as tc:
        with (
            tc.tile_pool(name="const", bufs=1) as const,
            tc.tile_pool(name="kq", bufs=2) as kqp,
            tc.tile_pool(name="at", bufs=4) as atp,
            tc.tile_pool(name="ost", bufs=4) as ostp,
            tc.tile_pool(name="ps_s", bufs=2, space="PSUM") as ps_s,
            tc.tile_pool(name="ps_av", bufs=1, space="PSUM") as ps_av,
            tc.tile_pool(name="ps_den", bufs=1, space="PSUM") as ps_den,
            tc.tile_pool(name="ps_bg", bufs=2, space="PSUM") as ps_bg,
        ):
            # ---- constants / big residents -------------------------------
            xT = const.tile([128, CT, N], BF16)
            wq = const.tile([128, CT, G], BF16)
            wk = const.tile([128, CT, G], BF16)
            wv = const.tile([128, CT, G], BF16)
            wp = const.tile([128, GP, C], BF16)
            bq = const.tile([128, GP], F32)
            bk = const.tile([128, GP], F32)
            ones_col = const.tile([128, 1], BF16)
            e0_blk = const.tile([128, D], BF16)
            e32_blk = const.tile([128, D], BF16)
            v64 = const.tile([128, KT, 8, D], BF16)  # this core's 8 heads
            ao = const.tile([128, NPAIR, N], BF16)
            pav_sb = const.tile([128, NPASS, 512], BF16)
            rcp_sb = const.tile([33, NPASS, 512], BF16)
            # proj partial staging: pairs 0-2 contribution per (qc, od),
            # [128 od-dims, qc, od, 512 queries]
            prt_sb = const.tile([128, NQC, CT, 512], BF16)

            nc.vector.memset(ones_col[:], 1.0)
            nc.vector.memset(e0_blk[:], 0.0)
            nc.vector.memset(e32_blk[:], 0.0)
            nc.vector.memset(e0_blk[0:1, :], 1.0)
            nc.vector.memset(e32_blk[32:33, :], 1.0)

            # DMA startup order == prologue consumption order, using few BIG
            # multi-kc descriptors (per-issue overhead on the sync queue is
            # ~650 ns; 32 small issues would gate the prologue). Priority:
            # prologue inputs (x 0-1023, pair-0 K/Q weights, V weights),
            # then x 1024-2047 (vb needs it by kt 8 of pass 0), then the
            # remaining weights.
            nc.sync.dma_start(out=xT[:, :, 0:512], in_=xT_r[:, :, 0:512])
            nc.sync.dma_start(out=wk[:, :, 0:128], in_=wk_r[:, :, 0:128])
            nc.sync.dma_start(out=wv[:, :, :], in_=wv_r[:, :, :])
            nc.sync.dma_start(out=bk[:], in_=bk_ext[:].rearrange("(o p) 1 -> p o", p=128))
            nc.sync.dma_start(out=bq[:], in_=bq_ext[:].rearrange("(o p) 1 -> p o", p=128))
            nc.sync.dma_start(out=wq[:, :, 0:128], in_=wq_r[:, :, 0:128])
            nc.sync.dma_start(out=xT[:, :, 512:1024], in_=xT_r[:, :, 512:1024])
            nc.sync.dma_start(out=xT[:, :, 1024:1536], in_=xT_r[:, :, 1024:1536])
            nc.sync.dma_start(out=xT[:, :, 1536:N], in_=xT_r[:, :, 1536:N])
            nc.sync.dma_start(out=wk[:, :, 128:G], in_=wk_r[:, :, 128:G])
            nc.sync.dma_start(out=wq[:, :, 128:G], in_=wq_r[:, :, 128:G])
            nc.sync.dma_start(out=wp[:, :, :], in_=wp_r[:, :, :])

            # dens psum rows 1-31 are read by the batched reciprocal but never
            # written by the M=1 denominator matmuls; preset once to 1.0.
            dens_init = ps_den.tile([128, 512], F32, name="dens", tag="dens")
            nc.vector.memset(dens_init[0:33, :], 1.0)

            # ---- background work: fine-grained chunk generators ----------
            def gen_k(mt, dst, t0, t1):
                """K projection for pair mt, token chunks [t0, t1)."""
                for t in range(t0, t1):
                    p = ps_bg.tile([128, 512], F32, tag="bg")
                    for kc in range(CT):
                        nc.tensor.matmul(
                            p[:],
                            lhsT=wk[:, kc, mt * 128 : (mt + 1) * 128],
                            rhs=xT[:, kc, t * 512 : (t + 1) * 512],
                            start=(kc == 0),
                            stop=(kc == CT - 1),
                            skip_group_check=True,
                        )
                        yield
                    nc.vector.tensor_tensor(
                        dst[:, t * 512 : (t + 1) * 512],
                        p[:],
                        bk[:, mt : mt + 1].to_broadcast([128, 512]),
                        mybir.AluOpType.add,
                    )

            def gen_q(mt, dst, t0, t1):
                """Q projection for pair mt, query chunks [t0, t1)."""
                for t in range(t0, t1):
                    p = ps_bg.tile([128, 512], F32, tag="bg")
                    for kc in range(CT):
                        nc.tensor.matmul(
                            p[:],
                            lhsT=wq[:, kc, mt * 128 : (mt + 1) * 128],
                            rhs=xT[:, kc, t * 512 : (t + 1) * 512],
                            start=(kc == 0),
                            stop=(kc == CT - 1),
                            skip_group_check=True,
                        )
                        yield
                    nc.vector.tensor_tensor(
                        dst[:, t * 512 : (t + 1) * 512],
                        p[:],
                        bq[:, mt : mt + 1].to_broadcast([128, 512]),
                        mybir.AluOpType.add,
                    )

            def gen_v(tt0, tt1):
                """V projection (all 8 heads), key tiles [tt0, tt1)."""
                for tt in range(tt0, tt1):
                    p = ps_bg.tile([128, 512], F32, tag="bg")
                    for kc in range(CT):
                        nc.tensor.matmul(
                            p[:],
                            lhsT=xT[:, kc, tt * 128 : (tt + 1) * 128],
                            rhs=wv[:, kc, :],
                            start=(kc == 0),
                            stop=(kc == CT - 1),
                            skip_group_check=True,
                        )
                        yield
                    nc.vector.tensor_copy(
                        v64[:, tt, :, :],
                        p[:].rearrange("p (h d) -> p h d", d=D),
                    )

            def gen_norm(ps):
                """Normalize pass ps=(mt, qc): broadcast 1/den, scale pav -> ao."""
                mt, qc = ps // NQC, ps % NQC
                pbc = ps_bg.tile([128, 512], F32, tag="bg")
                nc.tensor.matmul(
                    pbc[0:D, :], lhsT=e0_blk[0:33, :], rhs=rcp_sb[:, ps, :],
                    start=True, stop=True, skip_group_check=True,
                )
                yield
                nc.tensor.matmul(
                    pbc[D:128, :], lhsT=e32_blk[0:33, :], rhs=rcp_sb[:, ps, :],
                    start=True, stop=True,
                    tile_position=(0, D), skip_group_check=True,
                )
                yield
                nc.vector.tensor_tensor(
                    ao[:, mt, qc * 512 : (qc + 1) * 512],
                    pbc[:],
                    pav_sb[:, ps, :],
                    mybir.AluOpType.mult,
                )

            def gen_proj_part(qc):
                """Pairs 0-2 of the output projection for query chunk qc,
                staged to SBUF bf16 (needs n(qc), n(4+qc), n(8+qc))."""
                for od in range(CT):
                    p = ps_bg.tile([128, 512], F32, tag="bg")
                    for mt in range(NPAIR - 1):
                        nc.tensor.matmul(
                            p[:],
                            lhsT=wp[:, mt, od * 128 : (od + 1) * 128],
                            rhs=ao[:, mt, qc * 512 : (qc + 1) * 512],
                            start=(mt == 0),
                            stop=(mt == NPAIR - 2),
                            skip_group_check=True,
                        )
                        yield
                    nc.vector.tensor_copy(prt_sb[:, qc, od, :], p[:])

            def gen_proj_fin(qc, wide=False):
                """Last pair (mt=3) + staged partial -> out (needs n(12+qc)).

                wide=True (tail only): od-pairs through the (then-idle)
                scores psum pool — halves the MM->TT->DMA sem hops that
                pace the tail's shallow pipeline."""
                if wide:
                    for od in range(0, CT, 2):
                        p = ps_s.tile([128, 2, 512], F32, tag="pss")
                        for j in range(2):
                            nc.tensor.matmul(
                                p[:, j, :],
                                lhsT=wp[:, NPAIR - 1, (od + j) * 128 : (od + j + 1) * 128],
                                rhs=ao[:, NPAIR - 1, qc * 512 : (qc + 1) * 512],
                                start=True, stop=True,
                                skip_group_check=True,
                            )
                            yield
                        o_st = ostp.tile([128, 2, 512], BF16, tag="ost2")
                        nc.vector.tensor_tensor(
                            o_st[:],
                            p[:],
                            prt_sb[:, qc, od : od + 2, :],
                            mybir.AluOpType.add,
                        )
                        nc.sync.dma_start(
                            out=out_r[:, od : od + 2, qc * 512 : (qc + 1) * 512],
                            in_=o_st[:],
                        )
                    return
                for od in range(CT):
                    p = ps_bg.tile([128, 512], F32, tag="bg")
                    nc.tensor.matmul(
                        p[:],
                        lhsT=wp[:, NPAIR - 1, od * 128 : (od + 1) * 128],
                        rhs=ao[:, NPAIR - 1, qc * 512 : (qc + 1) * 512],
                        start=True, stop=True,
                        skip_group_check=True,
                    )
                    yield
                    o_st = ostp.tile([128, 512], BF16, tag="ost")
                    nc.vector.tensor_tensor(
                        o_st[:],
                        p[:],
                        prt_sb[:, qc, od, :],
                        mybir.AluOpType.add,
                    )
                    nc.sync.dma_start(
                        out=out_r[:, od, qc * 512 : (qc + 1) * 512], in_=o_st[:]
                    )

            # background queue machinery
            bg_queue = []
            bg_done = set()

            def bg_pump(n):
                done = 0
                while done < n and bg_queue:
                    try:
                        next(bg_queue[0][1])
                        done += 1
                    except StopIteration:
                        bg_done.add(bg_queue.pop(0)[0])

            def bg_require(*names):
                # pump MINIMALLY: an overshooting pump floods the in-order
                # PE queue with background matmuls ahead of the exp-critical
                # scores that follow this require in program order.
                while bg_queue and not all(n in bg_done for n in names):
                    bg_pump(1)

            def bg_drain():
                while bg_queue:
                    bg_pump(1 << 30)

            kq_tiles = {}

            def enqueue_pair(mt):
                kTn = kqp.tile([128, N], BF16, tag="kT")
                qTn = kqp.tile([128, N], BF16, tag="qT")
                kq_tiles[mt] = (kTn, qTn)
                bg_queue.append((f"k{mt}a", gen_k(mt, kTn, 0, 2)))
                bg_queue.append((f"q{mt}0", gen_q(mt, qTn, 0, 1)))
                bg_queue.append((f"k{mt}b", gen_k(mt, kTn, 2, 4)))
                bg_queue.append((f"q{mt}1", gen_q(mt, qTn, 1, 2)))
                bg_queue.append((f"q{mt}2", gen_q(mt, qTn, 2, 3)))
                bg_queue.append((f"q{mt}3", gen_q(mt, qTn, 3, 4)))

            # ---- prologue: order matches DMA arrival ---------------------
            # Only V key-tiles 0-3 are required up front; the rest of V and
            # K0's second half stream in as pass 0 consumes them.
            kT0 = kqp.tile([128, N], BF16, tag="kT")
            qT0 = kqp.tile([128, N], BF16, tag="qT")
            kq_tiles[0] = (kT0, qT0)
            bg_queue.append(("k0a0", gen_k(0, kT0, 0, 1)))
            bg_queue.append(("q00", gen_q(0, qT0, 0, 1)))
            bg_queue.append(("va0", gen_v(0, 4)))
            bg_require("k0a0", "q00", "va0")
            # consumption order == FIFO order == DMA arrival order
            bg_queue.append(("k0a1", gen_k(0, kT0, 1, 2)))
            bg_queue.append(("va1", gen_v(4, 8)))
            bg_queue.append(("k0b", gen_k(0, kT0, 2, 4)))
            bg_queue.append(("vb0", gen_v(8, 12)))
            bg_queue.append(("vb1", gen_v(12, KT)))
            bg_queue.append(("q01", gen_q(0, qT0, 1, 2)))
            bg_queue.append(("q02", gen_q(0, qT0, 2, 3)))
            bg_queue.append(("q03", gen_q(0, qT0, 3, 4)))

            # ---- attention ----------------------------------------------
            # 3/step spreads the ~560 background slots nearly evenly across
            # passes (~24/pass), keeping per-pass PE work under the ACT exp
            # floor; 5/step drains the queue in the first two passes of each
            # pair-block, making those passes PE-bound (~22us vs 16.7us).
            BG_PER_KT = 3

            # state carried across the hoisted pass boundary
            pending = {}  # (ps) -> {kt: at_tile} for hoisted scores

            def make_scores_exp(kTp, qTp, qc, at_tiles):
                def scores_exp(kt):
                    pss = ps_s.tile([128, 2, 512], F32, name="pss", tag="pss")
                    nc.tensor.matmul(
                        pss[:, 0, :],
                        lhsT=kTp[0:D, kt * 128 : (kt + 1) * 128],
                        rhs=qTp[0:D, qc * 512 : (qc + 1) * 512],
                        start=True, stop=True, skip_group_check=True,
                    )
                    nc.tensor.matmul(
                        pss[:, 1, :],
                        lhsT=kTp[D:128, kt * 128 : (kt + 1) * 128],
                        rhs=qTp[D:128, qc * 512 : (qc + 1) * 512],
                        start=True, stop=True, skip_group_check=True,
                    )
                    at = atp.tile([128, 2, 512], BF16, tag="at")
                    nc.scalar.activation(
                        at[:], pss[:],
                        mybir.ActivationFunctionType.Exp, scale=float(SCALE),
                    )
                    at_tiles[kt] = at

                return scores_exp

            for mt in range(NPAIR):
                if mt + 1 < NPAIR:
                    enqueue_pair(mt + 1)
                kTp, qTp = kq_tiles[mt]
                hl = 2 * mt
                for qc in range(NQC):
                    ps = NQC * mt + qc
                    if ps == 0:
                        bg_require("k0a0", "q00", "va0")
                    elif qc == 0:
                        bg_require(f"k{mt}a", f"q{mt}0", "vb1")
                    else:
                        bg_require(f"q{mt}{qc}")
                    pav = ps_av.tile([128, 512], F32, name="pav", tag="pav")
                    dens = ps_den.tile([128, 512], F32, name="dens", tag="dens")

                    at_tiles = pending.pop(ps, {})
                    scores_exp = make_scores_exp(kTp, qTp, qc, at_tiles)

                    def av_dens(kt):
                        at = at_tiles[kt]
                        first, last = kt == 0, kt == KT - 1
                        nc.tensor.matmul(
                            pav[0:D, :],
                            lhsT=v64[:, kt, hl, :],
                            rhs=at[:, 0, :],
                            start=first, stop=last,
                            skip_group_check=True,
                        )
                        nc.tensor.matmul(
                            pav[D:128, :],
                            lhsT=v64[:, kt, hl + 1, :],
                            rhs=at[:, 1, :],
                            start=first, stop=last,
                            tile_position=(0, D),
                            skip_group_check=True,
                        )

                    def dens_mm(kt):
                        at = at_tiles.pop(kt)
                        first, last = kt == 0, kt == KT - 1
                        nc.tensor.matmul(
                            dens[0:1, :],
                            lhsT=ones_col[:],
                            rhs=at[:, 0, :],
                            start=first, stop=last,
                            skip_group_check=True,
                        )
                        nc.tensor.matmul(
                            dens[32:33, :],
                            lhsT=ones_col[:],
                            rhs=at[:, 1, :],
                            start=first, stop=last,
                            tile_position=(0, 32),
                            skip_group_check=True,
                        )

                    if 0 not in at_tiles:
                        scores_exp(0)
                        scores_exp(1)
                    # ps 0: absorb the V/K prologue flood. ps 13-15: drain
                    # the projection finishers in-pass (a tail chunk costs
                    # ~3us serialized vs ~0.2us overlapped); passes 10-12
                    # stay at 3 — their proj-partial chunks are DVE-copy
                    # heavy and a faster pump there re-creates WAR stalls.
                    bg_per_kt = 12 if ps == 0 else (5 if ps >= 13 else BG_PER_KT)
                    for kt2 in range(0, KT, 2):
                        if ps == 0 and kt2 == 2:
                            # scores(4/5) read kT tokens 512-1023
                            bg_require("k0a1")
                        if kt2 + 2 < KT:
                            if kt2 + 2 == 8 and qc == 0:
                                bg_require(f"k{mt}b")
                            scores_exp(kt2 + 2)
                            scores_exp(kt2 + 3)
                        elif ps + 1 < NPASS:
                            # hoist next pass's first two scores+exp
                            nmt, nqc = (ps + 1) // NQC, (ps + 1) % NQC
                            if nqc == 0:
                                bg_require(f"k{nmt}a", f"q{nmt}0")
                            else:
                                bg_require(f"q{nmt}{nqc}")
                            nkT, nqT = kq_tiles[nmt]
                            nat = {}
                            nse = make_scores_exp(nkT, nqT, nqc, nat)
                            nse(0)
                            nse(1)
                            pending[ps + 1] = nat
                        if ps == 0:
                            # progressive V availability: these requires only
                            # protect av_dens below — emitted AFTER scores so
                            # the inline V-generation flood cannot block the
                            # exp-critical scores in the in-order PE queue.
                            if kt2 == 4:
                                bg_require("va1")
                            elif kt2 == 8:
                                bg_require("vb0")
                            elif kt2 == 12:
                                bg_require("vb1")
                        # attention first, bg last within each step: a bg
                        # matmul stalled on a psum WAR (DVE drain) must not
                        # sit ahead of the exp-critical attention matmuls in
                        # the in-order PE queue. av before dens: dens(0)
                        # waits on the den-bank copy at a pass boundary.
                        av_dens(kt2)
                        av_dens(kt2 + 1)
                        dens_mm(kt2)
                        dens_mm(kt2 + 1)
                        bg_pump(bg_per_kt)
                    if qc == NQC - 1:
                        kq_tiles.pop(mt)
                    # ---- pass end: stash pav/den on DVE ------------------
                    # copy_pav first (av(0) of the next pass WARs on it;
                    # dens(0) comes later in the PE queue), copy_den next,
                    # and the slow 3.3us reciprocal of pass ps-1 LAST — it
                    # is only consumed by n(ps-1) a full pass later, and
                    # must never sit ahead of the copies in the DVE queue.
                    nc.vector.tensor_copy(pav_sb[:, ps, :], pav[:])
                    nc.vector.tensor_copy(rcp_sb[:, ps, :], dens[0:33, :])
                    if 1 <= ps <= 14:
                        with nc.allow_low_precision("softmax denominator bf16"):
                            nc.vector.reciprocal(
                                rcp_sb[:, ps - 1, :], rcp_sb[:, ps - 1, :]
                            )
                        bg_queue.append((f"n{ps - 1}", gen_norm(ps - 1)))
                        psn = ps - 1
                        if psn >= 8 and psn < 12:
                            # n(8+qc') emitted -> pairs 0-2 of chunk qc' ready
                            bg_queue.append((f"pp{psn - 8}", gen_proj_part(psn - 8)))
                        if psn >= 12:
                            bg_queue.append((f"pf{psn - 12}", gen_proj_fin(psn - 12)))
                    if ps == 14:
                        # endgame: reciprocal pass 14 immediately (not lagged)
                        # so n(14) and proj-fin(2) execute DURING pass 15
                        # instead of spilling into the tail.
                        with nc.allow_low_precision("softmax denominator bf16"):
                            nc.vector.reciprocal(rcp_sb[:, 14, :], rcp_sb[:, 14, :])
                        bg_queue.append(("n14", gen_norm(14)))
                        bg_queue.append(("pf2", gen_proj_fin(2)))

            # ---- tail: last normalize + last projection finisher ---------
            with nc.allow_low_precision("softmax denominator bf16"):
                nc.vector.reciprocal(
                    rcp_sb[:, NPASS - 1, :], rcp_sb[:, NPASS - 1, :]
                )
            bg_queue.append((f"n{NPASS - 1}", gen_norm(NPASS - 1)))
            bg_queue.append(("pf3", gen_proj_fin(3, wide=True)))
            bg_drain()

    _split_sync_waits(nc)
    return nc


_CACHED_NC = None


def _get_nc():
    global _CACHED_NC
    if _CACHED_NC is None:
        _CACHED_NC = build()
    return _CACHED_NC


def make_in_maps(x, w_qkv, b_qkv, w_proj, b_proj):
    bf = ml_dtypes.bfloat16
    xTs = []
    for b in range(B):
        xTs.append(np.ascontiguousarray(x[b].T.astype(bf)))  # [C, N]
    in_maps = []
    for core in range(NCORES):
        b = core // 2
        g = core % 2
        sl = slice(g * G, (g + 1) * G)
        in_maps.append(
            {
                "xT": xTs[b],
                "w_q": np.ascontiguousarray(w_qkv[:, sl].astype(bf)),
                "w_k": np.ascontiguousarray(w_qkv[:, C + g * G : C + (g + 1) * G].astype(bf)),
                "w_v": np.ascontiguousarray(w_qkv[:, 2 * C + g * G : 2 * C + (g + 1) * G].astype(bf)),
                "w_p": np.ascontiguousarray(w_proj[sl, :].astype(bf)),
                "b_q": np.ascontiguousarray(b_qkv[sl].reshape(G, 1).astype(np.float32)),
                "b_k": np.ascontiguousarray(
                    b_qkv[C + g * G : C + (g + 1) * G].reshape(G, 1).astype(np.float32)
                ),
            }
        )
    return in_maps


def run(x, w_qkv, b_qkv, w_proj, b_proj, trace=False, **spmd_kwargs):
    nc = _get_nc()
    in_maps = make_in_maps(x, w_qkv, b_qkv, w_proj, b_proj)
    res = run_bass_kernel_spmd(
        nc, in_maps, core_ids=list(range(NCORES)), trace=trace, **spmd_kwargs
    )
    # host-side reduction of the two head-half partials + folded bias
    b_v = b_qkv[2 * C : 3 * C].astype(np.float32)
    bias = b_proj.astype(np.float32) + b_v @ w_proj.astype(np.float32)  # [C]
    out = np.empty((B, N, C), dtype=np.float32)
    for b in range(B):
        yT = res.results[2 * b]["out"].astype(np.float32) + res.results[
            2 * b + 1
        ]["out"].astype(np.float32)  # [C, N]
        out[b] = yT.T + bias
    return out, res


def kernel(x, w_qkv, b_qkv, w_proj, b_proj):
    x = np.asarray(x, dtype=np.float32)
    w_qkv = np.asarray(w_qkv, dtype=np.float32)
    b_qkv = np.asarray(b_qkv, dtype=np.float32)
    w_proj = np.asarray(w_proj, dtype=np.float32)
    b_proj = np.asarray(b_proj, dtype=np.float32)
    out, _ = run(x, w_qkv, b_qkv, w_proj, b_proj, trace=False)
    return out
